# revision 1
# baseline (speedup 1.0000x reference)
"""YOLO-head decode (nms_detection) on Trainium2.

Data-parallel over the batch dim: 16 batches -> 2 per core x 8 NeuronCores.
Each core decodes its slice of the three feature maps fully on-chip:
  box:   x1=(b0-b2/2)*W ; y1=(b1-b3/2)*H ; x2=(x1+b2/2)*W ; y2=(y1+b3/2)*H
  conf:  sigmoid(b4) ; cls: sigmoid(b5:85)
and writes the packed [B, 25200, 85] nms input.

Layout trick: the three maps are concatenated host-side into one packed
[2, 25200, 85] per-core payload (already the packed output order), then
viewed as 126 partitions x 400 cells x 85 channels.  Every fm/batch
boundary (19200, 24000, 25200 cells per batch) is a multiple of 400, so
each partition holds cells of exactly one feature map and the per-fm box
scale W (=H) is a per-partition scalar, built on-chip from iota.  Every
tile is an identical full-width [126, 25*85] slab whose loads/stores are
~1.07 MB contiguous DMAs, and the output needs no host-side reordering.

Sync discipline (this walrus accepts ONE sync-wait per instruction; a
JSON-level legalizer splits any multi-wait instruction into single-wait
Drains).  16 chunks; loads go through SWDGE (Pool-issued) and stores
through HWDGE (SP-issued) so the two directions stream on independent
issue paths.  Per tile:
  load(DMA) -> extract box cols (ACT, waits load)
  -> sigmoid whole tile in place (ACT, self-wait only)
  -> box math on scratch (DVE, waits ACT extract)
  -> 1-elem ACT probe of last DVE result (absorbs the DVE wait)
  -> writeback box cols (ACT, self-wait only)
  -> store (DMA, waits ACT; sigmoid covered every byte of the tile)
"""

import json

import numpy as np

_N_CORES = 8
_B_PER_CORE = 2
_D = 85            # 5 + 80 channels per cell
_P = 126           # partitions per tile
_KC = 400          # cells per partition per core (126*400 = 50400 = 2*25200)
_NCHUNK = 16       # k-chunks
_K = _KC // _NCHUNK          # cells per partition per tile
_F = _K * _D                 # f32 per partition per tile
# Front-tapered schedule: small first tiles prime the store path ~3 us
# sooner (sim 63.2 us vs 64.3 us uniform); sums to _KC.
_DEFAULT_CHUNKS = [8, 16, 24] + [27] * 12 + [28]
_N_TOT = 25200

_state = {}


def _build(repeat=1, nchunk=_NCHUNK, load_engine="gpsimd", io_bufs=14,
           chunks=None, fused=False, store_engines=("sync",),
           load_engines=None, kc=_KC):
    import concourse.bass as bass
    import concourse.mybir as mybir
    from concourse.tile import TileContext

    if chunks is None:
        if kc == _KC and nchunk == _NCHUNK:
            chunks = list(_DEFAULT_CHUNKS)
        else:
            chunks = [kc // nchunk] * nchunk
    assert sum(chunks) == kc, chunks
    Kmax = max(chunks)

    f32 = mybir.dt.float32
    i32 = mybir.dt.int32
    MUL = mybir.AluOpType.mult
    ADD = mybir.AluOpType.add
    LE = mybir.AluOpType.is_le
    GE = mybir.AluOpType.is_ge
    SIG = mybir.ActivationFunctionType.Sigmoid

    nc = bass.Bass()
    x = nc.dram_tensor("x", [_P, kc * _D], f32, kind="ExternalInput")
    out = nc.dram_tensor("out", [_P, kc * _D], f32, kind="ExternalOutput")

    with TileContext(nc) as tc:
        with (
            tc.tile_pool(name="const", bufs=1) as cp,
            tc.tile_pool(name="io", bufs=min(nchunk, io_bufs)) as io_pool,
            tc.tile_pool(name="scratch", bufs=min(nchunk, 6)) as sp,
        ):
            # Per-partition box scale. Rows (mod 63): 0:48 -> fm0 (80),
            # 48:60 -> fm1 (40), 60:63 -> fm2 (20); batch 1 mirrors at row 63.
            # scale = 20 + 40*(q<=47) + 20*(q<=59), q = p - 63*(p>=63).
            idx = cp.tile([_P, 1], i32, name="idx")
            nc.gpsimd.iota(idx[:], pattern=[[1, 1]], base=0, channel_multiplier=1)
            idxf = cp.tile([_P, 1], f32, name="idxf")
            nc.vector.tensor_copy(out=idxf[:], in_=idx[:])
            a = cp.tile([_P, 1], f32, name="a")
            qf = cp.tile([_P, 1], f32, name="qf")
            c1 = cp.tile([_P, 1], f32, name="c1")
            c2 = cp.tile([_P, 1], f32, name="c2")
            vc = cp.tile([_P, 1], f32, name="vc")
            nc.vector.tensor_scalar(a[:], idxf[:], 62.5, None, op0=GE)
            nc.vector.scalar_tensor_tensor(qf[:], a[:], -63.0, idxf[:], op0=MUL, op1=ADD)
            nc.vector.tensor_scalar(c1[:], qf[:], 47.5, None, op0=LE)
            nc.vector.tensor_scalar(c2[:], qf[:], 59.5, None, op0=LE)
            nc.vector.tensor_scalar_mul(c2[:], c2[:], 20.0)
            nc.vector.scalar_tensor_tensor(vc[:], c1[:], 40.0, c2[:], op0=MUL, op1=ADD)
            nc.vector.tensor_scalar_add(vc[:], vc[:], 20.0)

            offs = [0]
            for K in chunks:
                offs.append(offs[-1] + K)
            for ci in [c for _ in range(repeat) for c in range(len(chunks))]:
                K = chunks[ci]
                F = K * _D
                src = x[:, offs[ci] * _D:offs[ci + 1] * _D]
                dst = out[:, offs[ci] * _D:offs[ci + 1] * _D]
                st_eng = store_engines[ci % len(store_engines)]
                ld_eng = (load_engines[ci % len(load_engines)]
                          if load_engines else load_engine)

                tl = io_pool.tile([_P, F], f32, tag="io", name="io",
                                  padded_shape=[_P, Kmax * _D])
                getattr(nc, ld_eng).dma_start(out=tl[:], in_=src)
                v = tl.rearrange("p (k c) -> p k c", c=_D)
                if fused:
                    # box math straight out of / into the io tile; strided
                    # sigmoid on conf+cls only.  Multi-wait DMAs are made
                    # legal by the BIR legalizer.
                    h2 = sp.tile([_P, K], f32, tag="h2", name="h2")
                    h3 = sp.tile([_P, K], f32, tag="h3", name="h3")
                    u = sp.tile([_P, K], f32, tag="u", name="u")
                    q = sp.tile([_P, K], f32, tag="q", name="q")
                    t1 = sp.tile([_P, K], f32, tag="t1", name="t1")
                    t2 = sp.tile([_P, K], f32, tag="t2", name="t2")
                    w = vc[:, :]
                    c0, c1, c2, c3 = (v[:, :, j] for j in range(4))
                    nc.vector.tensor_scalar_mul(h2[:], c2, 0.5)
                    nc.vector.tensor_scalar_mul(h3[:], c3, 0.5)
                    nc.vector.tensor_sub(u[:], c0, h2[:])
                    nc.vector.tensor_sub(q[:], c1, h3[:])
                    nc.vector.tensor_scalar_mul(c0, u[:], w)  # x1
                    nc.vector.tensor_scalar_mul(c1, q[:], w)  # y1
                    nc.vector.scalar_tensor_tensor(t1[:], u[:], w, h2[:], op0=MUL, op1=ADD)
                    nc.vector.tensor_scalar_mul(c2, t1[:], w)  # x2
                    nc.vector.scalar_tensor_tensor(t2[:], q[:], w, h3[:], op0=MUL, op1=ADD)
                    nc.vector.tensor_scalar_mul(c3, t2[:], w)  # y2
                    nc.scalar.activation(v[:, :, 4:_D], v[:, :, 4:_D], SIG)
                    getattr(nc, st_eng).dma_start(out=dst, in_=tl[:])
                    continue
                h2 = sp.tile([_P, K], f32, tag="h2", name="h2")
                h3 = sp.tile([_P, K], f32, tag="h3", name="h3")
                u = sp.tile([_P, K], f32, tag="u", name="u")
                q = sp.tile([_P, K], f32, tag="q", name="q")
                t1 = sp.tile([_P, K], f32, tag="t1", name="t1")
                t2 = sp.tile([_P, K], f32, tag="t2", name="t2")
                bxr = sp.tile([_P, 4 * K], f32, tag="bxr", name="bxr")
                bx = sp.tile([_P, 4 * K], f32, tag="bx", name="bx")
                pb = sp.tile([_P, 1], f32, tag="pb", name="pb")
                rv = bxr.rearrange("p (k c) -> p k c", c=4)
                bv = bx.rearrange("p (k c) -> p k c", c=4)

                # ACT: extract raw box cols, then sigmoid the whole tile
                nc.scalar.copy(rv[:], v[:, :, 0:4])
                nc.scalar.activation(tl[:, :], tl[:, :], SIG)

                # DVE: box decode (bit-faithful op order vs the reference)
                r0, r1, r2, r3 = (rv[:, :, j] for j in range(4))
                w = vc[:, :]
                nc.vector.tensor_scalar_mul(h2[:], r2, 0.5)
                nc.vector.tensor_scalar_mul(h3[:], r3, 0.5)
                nc.vector.tensor_sub(u[:], r0, h2[:])
                nc.vector.tensor_sub(q[:], r1, h3[:])
                nc.vector.tensor_scalar_mul(bv[:, :, 0], u[:], w)  # x1
                nc.vector.tensor_scalar_mul(bv[:, :, 1], q[:], w)  # y1
                nc.vector.scalar_tensor_tensor(t1[:], u[:], w, h2[:], op0=MUL, op1=ADD)
                nc.vector.tensor_scalar_mul(bv[:, :, 2], t1[:], w)  # x2
                nc.vector.scalar_tensor_tensor(t2[:], q[:], w, h3[:], op0=MUL, op1=ADD)
                nc.vector.tensor_scalar_mul(bv[:, :, 3], t2[:], w)  # y2

                # ACT: probe last DVE result (absorbs DVE wait), writeback
                nc.scalar.copy(pb[:], bv[:, 0:1, 3])
                nc.scalar.copy(v[:, :, 0:4], bv[:])

                getattr(nc, st_eng).dma_start(out=dst, in_=tl[:])

    return nc


def _split_multiwait_bir(bir_json):
    """Walrus codegen accepts a single sync-wait per instruction, but Tile's
    kernel-tail drain carries one wait per logical processor.  Split any
    multi-wait instruction into a chain of single-wait Drains on the same
    engine, keeping the last wait on the original instruction."""
    m = json.loads(bir_json)
    n = [0]

    def fix_block(b):
        insts = b.get("instructions") or []
        fixed = []
        for ins in insts:
            si = ins.get("sync_info") or {}
            waits = si.get("on_wait") or []
            if len(waits) > 1:
                for wt in waits[:-1]:
                    n[0] += 1
                    fixed.append({
                        "debug": ins.get("debug", 0),
                        "engine": ins["engine"],
                        "ins": [],
                        "name": f"I-waitsplit-{n[0]}",
                        "opcode": "Drain",
                        "outs": [],
                        "sync_info": {"on_update": [], "on_wait": [wt]},
                    })
                si["on_wait"] = [waits[-1]]
            fixed.append(ins)
        if insts:
            b["instructions"] = fixed
        for sb in b.get("blocks") or []:
            fix_block(sb)

    for fn in m["functions"]:
        for b in fn["blocks"]:
            fix_block(b)
    return json.dumps(m).encode()


def _install_bir_legalizer():
    if _state.get("patched"):
        return
    import concourse.bass2jax as bass2jax
    from concourse.bass_utils import compile_bir_kernel as orig

    def patched(bir_json, tmpdir, neff_name="file.neff"):
        return orig(_split_multiwait_bir(bir_json), tmpdir, neff_name)

    bass2jax.compile_bir_kernel = patched
    _state["patched"] = True


def _get_nc():
    if "nc" not in _state:
        _state["nc"] = _build()
    return _state["nc"]


def _pack(fm0, fm1, fm2):
    B = fm0.shape[0]
    return np.concatenate(
        [
            fm0.reshape(B, -1, _D),
            fm1.reshape(B, -1, _D),
            fm2.reshape(B, -1, _D),
        ],
        axis=1,
    )


def _run_shards(fm0, fm1, fm2, **run_kwargs):
    from concourse.bass_utils import run_bass_kernel_spmd

    _install_bir_legalizer()
    nc = _get_nc()
    packed = _pack(fm0, fm1, fm2)  # [16, 25200, 85]
    in_maps = []
    for i in range(_N_CORES):
        s = packed[_B_PER_CORE * i:_B_PER_CORE * (i + 1)]
        in_maps.append({"x": s.reshape(_P, _KC * _D)})
    res = run_bass_kernel_spmd(nc, in_maps, list(range(_N_CORES)), **run_kwargs)
    out = np.concatenate(
        [r["out"].reshape(_B_PER_CORE, _N_TOT, _D) for r in res.results], axis=0
    )
    return out, res


def _direct_runner():
    """Direct shard_map runner over the prebuilt Bass module.  Equivalent to
    run_bass_kernel_spmd's axon path but feeds the packed full-batch array
    without the per-core split + re-concat, and keeps the (never-read,
    fully-overwritten) output buffers resident on device across calls
    instead of shipping 137 MB of zeros through axon per call."""
    if "direct" in _state:
        return _state["direct"]

    import jax
    import concourse.mybir as mybir
    from concourse.bass2jax import _bass_exec_p, partition_id_tensor
    from jax.sharding import Mesh, PartitionSpec, NamedSharding
    from jax.experimental.shard_map import shard_map

    _install_bir_legalizer()
    nc = _get_nc()
    partition_name = nc.partition_id_tensor.name if nc.partition_id_tensor else None
    out_avals, zero_outs = [], []
    for alloc in nc.m.functions[0].allocations:
        if not isinstance(alloc, mybir.MemoryLocationSet):
            continue
        if alloc.kind == "ExternalOutput":
            shape = tuple(alloc.tensor_shape)
            dtype = mybir.dt.np(alloc.dtype)
            out_avals.append(jax.core.ShapedArray(shape, dtype))
            zero_outs.append(np.zeros(shape, dtype))
    in_names = ["x", "out"]
    if partition_name is not None:
        in_names.append(partition_name)

    def _body(*args):
        operands = list(args)
        if partition_name is not None:
            operands.append(partition_id_tensor())
        return tuple(_bass_exec_p.bind(
            *operands, out_avals=tuple(out_avals), in_names=tuple(in_names),
            out_names=("out",), lowering_input_output_aliases=(),
            sim_require_finite=True, sim_require_nnan=True, nc=nc))

    devices = jax.devices()[:_N_CORES]
    assert len(devices) == _N_CORES
    mesh = Mesh(np.asarray(devices), ("core",))
    spec = PartitionSpec("core")
    sharded = jax.jit(shard_map(
        _body, mesh=mesh, in_specs=(spec, spec), out_specs=(spec,),
        check_rep=False))
    sh = NamedSharding(mesh, spec)
    dev_zeros = jax.device_put(
        np.zeros((_N_CORES * _P, _KC * _D), np.float32), sh)
    _state["direct"] = (sharded, dev_zeros)
    return _state["direct"]


def kernel(fm0, fm1, fm2, detection_targets=None, **_unused):
    fm0 = np.asarray(fm0, dtype=np.float32)
    fm1 = np.asarray(fm1, dtype=np.float32)
    fm2 = np.asarray(fm2, dtype=np.float32)
    try:
        packed = _pack(fm0, fm1, fm2).reshape(_N_CORES * _P, _KC * _D)
        sharded, dev_zeros = _direct_runner()
        (out,) = sharded(packed, dev_zeros)
        return np.asarray(out).reshape(_N_CORES * _B_PER_CORE, _N_TOT, _D)
    except Exception:
        _state.pop("direct", None)
        out, _ = _run_shards(fm0, fm1, fm2)
        return out

